# revision 18
# baseline (speedup 1.0000x reference)
"""BertBiAttention (3-modality co-attention) Trainium2 Bass kernel.

Sharding: data-parallel over batch B=8 across the 8 NeuronCores — each core
computes one batch element end-to-end (no collectives).

Per-core strategy (fp16 matmul inputs, fp32 accumulation):
  - Host pre-transposes/casts inputs: XT[m] = x_m.T  [D=1024, L=512] fp16;
    wq is split into per-head-half wqa/wqb [D, 512] so QTa/QTb chunks hold
    each head's 32 q-dims at partition offset 32*(h%4), matching KT chunks.
  - Projections on PE: QTa/QTb = wqa/wqb^T @ x^T, KT[m] = wk^T @ x^T,
    V[m] natural = x @ wv (chunks [128, 512]); the D=1024 contraction is
    split into two K=64 row groups (0 and 64) accumulating into separate
    PSUM banks so consecutive matmuls' LDWEIGHTS/streams overlap; the two
    banks are summed during evacuation (ACT copy + DVE add).
  - Scores computed once, as scores^T[k,q] chunks, with K_c=32 row-tiled
    matmuls 4-way packed across heads (row groups 0/32/64/96, separate
    banks) -> ACT exp(SCALE*s) -> fp16 expT.
  - Softmax denominators: ones-vector matmuls over expT (col-packed),
    gathered and PE-transposed to [q, head]; one reciprocal per quad.
  - probs output: expT blocks PE-transposed back to [q, k] layout in PSUM
    (fp16), then one DVE pass applies the per-partition reciprocal and
    widens to fp32; one consolidated 512 KiB DMA per head-half.
  - VC[branch] = per-head [va|vb] interleave of V, built by GPSIMD strided
    copies; ctx^T per head = single 4-matmul accumulation group
    [K=128, M=64, N=512] (even heads col-group 0, odd col-group 64, own
    banks); PE-transpose [64,128] blocks back to natural [q, d]; the
    normalization rides the PSUM evacuation as a per-partition scalar.
  - Quads (branch, head-quad) are software-pipelined: B-phase(i) is
    emitted before post-phase(i-1) so the PE works on quad i's scores
    while ACT computes quad i-1's exponentials.

Masks and biases are zero for this problem instance (per its input spec);
a numpy fallback handles any nonzero masks/biases just in case.
"""

import math
import os
import sys

import numpy as np

sys.path.insert(0, "/opt/trn_rl_repo")

from contextlib import ExitStack

import concourse.bass as bass
import concourse.tile as tile
from concourse import bacc, mybir
from concourse.bass import ts
from concourse.bass_utils import run_bass_kernel_spmd
from concourse.masks import make_identity

B, T, S = 8, 8, 64
L = T * S  # 512
D = 1024
H = 16
DH = 64  # q head size
DKV = 512  # k/v projection width (32 per head)
SCALE = 1.0 / math.sqrt(DH)
NCORES = 8

F16 = mybir.dt.float16
F32 = mybir.dt.float32
EXP = mybir.ActivationFunctionType.Exp

# (q_mod, a_mod, b_mod) per branch; reference returns
# (c[1], c[0], c[2], (p[1], p[0], p[2])) in this numbering.
BRANCHES = [(1, 0, 2), (0, 1, 2), (2, 0, 1)]

_COMPILED = None


def _build_kernel(tc, outs, ins):
    nc = tc.nc
    with ExitStack() as ctx:
        const = ctx.enter_context(tc.tile_pool(name="const", bufs=1))
        xpool = ctx.enter_context(tc.tile_pool(name="xpool", bufs=1))
        wpool = ctx.enter_context(tc.tile_pool(name="wpool", bufs=16))
        proj = ctx.enter_context(tc.tile_pool(name="proj", bufs=1))
        vcpool = ctx.enter_context(tc.tile_pool(name="vcpool", bufs=12))
        bpool = ctx.enter_context(tc.tile_pool(name="bpool", bufs=48))
        srpool = ctx.enter_context(tc.tile_pool(name="srpool", bufs=3))
        rpool = ctx.enter_context(tc.tile_pool(name="rpool", bufs=6))
        opool = ctx.enter_context(tc.tile_pool(name="opool", bufs=3))
        cpool = ctx.enter_context(tc.tile_pool(name="cpool", bufs=4))
        copool = ctx.enter_context(tc.tile_pool(name="copool", bufs=6))
        evpool = ctx.enter_context(tc.tile_pool(name="evpool", bufs=4))
        pspool = ctx.enter_context(tc.tile_pool(name="ps", bufs=8, space="PSUM"))

        ident = const.tile([128, 128], F16, name="ident", tag="ident")
        make_identity(nc, ident)
        ones = const.tile([128, 1], F16, name="ones", tag="ones")
        ident32 = const.tile([128, 128], F32, name="ident32", tag="ident32")
        make_identity(nc, ident32)
        nc.vector.memset(ones, 1.0)

        # ---- load XT chunks (resident); input loads trigger on gpsimd ----
        xt = [None] * 3  # xt[m][dc] : [128, 512] f16, rows = D dims dc*128..

        def _xt_load(m):
            if xt[m] is None:
                chunks = []
                for dc in range(8):
                    t = xpool.tile(
                        [128, L], F16, name=f"xt{m}_{dc}", tag=f"xt{m}_{dc}"
                    )
                    eng = nc.gpsimd if dc % 2 == 0 else nc.sync
                    eng.dma_start(out=t, in_=ins[f"xt{m}"][ts(dc, 128), :])
                    chunks.append(t)
                xt[m] = chunks

        # ---- projections (c-outer, dc-inner: 2 live banks, smooth evac) ----
        qta = [[None] * 4 for _ in range(3)]  # [m][hq] -> [128, 512] f16
        qtb = [[None] * 4 for _ in range(3)]
        kt = [[None] * 4 for _ in range(3)]  # K^T chunks
        vn = [[None] * 4 for _ in range(3)]  # V natural chunks [128 l, 512 dkv]

        def _wload(kind, m):
            tiles = []
            for dc in range(8):
                t = wpool.tile([128, DKV], F16, name=f"{kind}{m}_{dc}", tag="w")
                eng = nc.gpsimd if dc % 2 == 0 else nc.sync
                eng.dma_start(out=t, in_=ins[f"{kind}{m}"][ts(dc, 128), :])
                tiles.append(t)
            return tiles

        def _ksplit_mms(out_lo, out_hi, lhs_of, rhs_of):
            # contraction over 16 K=64 chunks: even halves -> row group 0
            # (bank out_lo), odd halves -> row group 64 (bank out_hi); the
            # alternating row groups let LDWEIGHTS/matmul streams overlap.
            for dc in range(8):
                st, sp = dc == 0, dc == 7
                nc.tensor.matmul(
                    out_lo, lhsT=lhs_of(dc, 0, 64), rhs=rhs_of(dc, 0, 64),
                    start=st, stop=sp, tile_position=(0, 0),
                )
                nc.tensor.matmul(
                    out_hi, lhsT=lhs_of(dc, 64, 128), rhs=rhs_of(dc, 64, 128),
                    start=st, stop=sp, tile_position=(64, 0),
                )

        def _evac_add(dst, p_lo, p_hi):
            tmp = evpool.tile([128, p_hi.shape[1]], F16, name="evtmp", tag="evtmp")
            nc.scalar.copy(tmp, p_hi)
            nc.vector.scalar_tensor_tensor(
                dst, p_lo, 1.0, tmp,
                op0=mybir.AluOpType.mult, op1=mybir.AluOpType.add,
            )

        def emit_q_proj(m):
            _xt_load(m)
            wa = _wload("wqa", m)
            wb = _wload("wqb", m)
            for c in range(4):
                qa_lo = pspool.tile([128, L], F32, name=f"qa_lo{m}_{c}", tag="ps")
                qa_hi = pspool.tile([128, L], F32, name=f"qa_hi{m}_{c}", tag="ps")
                qb_lo = pspool.tile([128, L], F32, name=f"qb_lo{m}_{c}", tag="ps")
                qb_hi = pspool.tile([128, L], F32, name=f"qb_hi{m}_{c}", tag="ps")
                _ksplit_mms(
                    qa_lo, qa_hi,
                    lambda dc, a, b: wa[dc][a:b, ts(c, 128)],
                    lambda dc, a, b: xt[m][dc][a:b, :],
                )
                _ksplit_mms(
                    qb_lo, qb_hi,
                    lambda dc, a, b: wb[dc][a:b, ts(c, 128)],
                    lambda dc, a, b: xt[m][dc][a:b, :],
                )
                ta = proj.tile([128, L], F16, name=f"qta{m}_{c}", tag=f"qta{m}_{c}")
                _evac_add(ta, qa_lo, qa_hi)
                qta[m][c] = ta
                tb = proj.tile([128, L], F16, name=f"qtb{m}_{c}", tag=f"qtb{m}_{c}")
                _evac_add(tb, qb_lo, qb_hi)
                qtb[m][c] = tb

        def emit_kv_proj(m):
            _xt_load(m)
            wk = _wload("wk", m)
            wv = _wload("wv", m)
            for c in range(4):
                k_lo = pspool.tile([128, L], F32, name=f"k_lo{m}_{c}", tag="ps")
                k_hi = pspool.tile([128, L], F32, name=f"k_hi{m}_{c}", tag="ps")
                v_lo = pspool.tile([128, DKV], F32, name=f"v_lo{m}_{c}", tag="ps")
                v_hi = pspool.tile([128, DKV], F32, name=f"v_hi{m}_{c}", tag="ps")
                _ksplit_mms(
                    k_lo, k_hi,
                    lambda dc, a, b: wk[dc][a:b, ts(c, 128)],
                    lambda dc, a, b: xt[m][dc][a:b, :],
                )
                _ksplit_mms(
                    v_lo, v_hi,
                    lambda dc, a, b: xt[m][dc][a:b, ts(c, 128)],
                    lambda dc, a, b: wv[dc][a:b, :],
                )
                tk = proj.tile([128, L], F16, name=f"kt{m}_{c}", tag=f"kt{m}_{c}")
                _evac_add(tk, k_lo, k_hi)
                kt[m][c] = tk
                tv = proj.tile([128, DKV], F16, name=f"vn{m}_{c}", tag=f"vn{m}_{c}")
                _evac_add(tv, v_lo, v_hi)
                vn[m][c] = tv

        # branch0 needs q:mod1, kv:mod0, kv:mod2 -> emit those first so
        # attention overlaps the remaining projections.
        emit_q_proj(1)
        emit_kv_proj(0)
        emit_kv_proj(2)
        emit_kv_proj(1)
        emit_q_proj(0)
        emit_q_proj(2)

        # ---- attention: software-pipelined quads across branches ----
        # quad index i = 4*br + hq; emit B-phase(i) then post-phase(i-1).
        vc_by_br = {}

        def emit_vc(br):
            qm, am, bm = BRANCHES[br]
            vcs = []
            for lc in range(4):
                v = vcpool.tile([128, D], F16, name=f"vc{br}_{lc}", tag="vc")
                vr = v.rearrange("p (h t j) -> p h t j", t=2, j=32)
                nc.gpsimd.tensor_copy(
                    vr[:, :, 0, :], vn[am][lc].rearrange("p (h j) -> p h j", j=32)
                )
                nc.gpsimd.tensor_copy(
                    vr[:, :, 1, :], vn[bm][lc].rearrange("p (h j) -> p h j", j=32)
                )
                vcs.append(v)
            vc_by_br[br] = vcs

        def emit_b_phase(i):
            br, hq = divmod(i, 4)
            qm, am, bm = BRANCHES[br]
            if hq == 0:
                emit_vc(br)
            expT = {}
            for kti in range(4):
                psb = []
                for hh in range(4):
                    off = 32 * hh
                    pb = pspool.tile(
                        [128, L], F32, name=f"psB{br}_{hq}_{kti}_{hh}", tag="ps"
                    )
                    nc.tensor.matmul(
                        pb,
                        lhsT=kt[am][hq][off : off + 32, ts(kti, 128)],
                        rhs=qta[qm][hq][off : off + 32, :],
                        start=True, stop=False, tile_position=(off, 0),
                    )
                    nc.tensor.matmul(
                        pb,
                        lhsT=kt[bm][hq][off : off + 32, ts(kti, 128)],
                        rhs=qtb[qm][hq][off : off + 32, :],
                        start=False, stop=True, tile_position=(off, 0),
                    )
                    psb.append(pb)
                for hh in range(4):
                    e = bpool.tile(
                        [128, L], F16, name=f"expT{br}_{hq}_{kti}_{hh}", tag="expT"
                    )
                    nc.scalar.activation(e, psb[hh], EXP, scale=SCALE)
                    expT[(hh, kti)] = e
            return expT

        def emit_post_phase(i, expT):
            br, hq = divmod(i, 4)
            qm, am, bm = BRANCHES[br]
            vc = vc_by_br[br]
            p_out = outs[f"p{br}"]
            c_out = outs[f"c{br}"]

            # 1) softmax denominators: sum_k expT via ones-matmuls
            sum_ps = []
            for hh in range(4):
                sp_t = pspool.tile([128, L], F32, name=f"psS{br}_{hq}_{hh}", tag="ps")
                sum_ps.append(sp_t)
            for kti in range(4):
                for hh in range(4):
                    nc.tensor.matmul(
                        sum_ps[hh][32 * hh : 32 * hh + 1, :],
                        lhsT=ones,
                        rhs=expT[(hh, kti)],
                        start=(kti == 0), stop=(kti == 3),
                        tile_position=(0, 32 * hh),
                    )
            # 2) gather rows (ACT) so PE isn't gated on the DVE queue
            srows4 = srpool.tile([128, L], F32, name=f"srows4{br}_{hq}", tag="srows4")
            for hh in range(4):
                nc.vector.tensor_copy(
                    srows4[32 * hh : 32 * hh + 1, :],
                    sum_ps[hh][32 * hh : 32 * hh + 1, :],
                )

            # 3) ctx^T matmuls (independent of sums -> keeps PE busy)
            pcs = {}
            for hp in range(2):
                for sub in range(2):
                    hh = 2 * hp + sub
                    h = 4 * hq + hh
                    lo = 64 * sub
                    pc = pspool.tile(
                        [128, L], F32, name=f"psC{br}_{hq}_{hh}", tag="ps"
                    )
                    pcs[hh] = pc
                    for kti in range(4):
                        nc.tensor.matmul(
                            pc[lo : lo + 64, :],
                            lhsT=vc[kti][:, ts(h, 64)],
                            rhs=expT[(hh, kti)],
                            start=(kti == 0), stop=(kti == 3),
                            tile_position=(0, lo),
                        )
            csbs = {}
            for hh in range(4):
                lo = 64 * (hh % 2)
                csb = cpool.tile([64, L], F16, name=f"csb{br}_{hq}_{hh}", tag="csb")
                nc.vector.tensor_copy(csb, pcs[hh][lo : lo + 64, :])
                csbs[hh] = csb

            # 4) sums transpose + reciprocal
            psT4 = pspool.tile([128, L], F32, name=f"psT4{br}_{hq}", tag="ps")
            for qt in range(4):
                nc.tensor.transpose(
                    psT4[:, ts(qt, 128)], srows4[:, ts(qt, 128)], ident32
                )
            # rc[:, 4*qt + hh] = 1 / sum for q-rows of tile qt, head hh
            rc = rpool.tile([128, 16], F32, name=f"rc{br}_{hq}", tag="rc")
            nc.vector.reciprocal(
                rc.rearrange("p (a e j) -> p a e j", a=4, e=4, j=1),
                psT4.rearrange("p (a e j) -> p a e j", a=4, j=32)[:, :, :, 0:1],
            )

            # 5) probs: PE-transpose expT blocks, normalize+evac, 1 DMA/head
            for hh in range(4):
                pn = opool.tile([128, 4 * L], F32, name=f"pn{br}_{hq}_{hh}", tag="pn")
                for qtp in range(2):
                    pPT = pspool.tile(
                        [128, 1024], F16, name=f"psPT{br}_{hq}_{hh}_{qtp}", tag="ps"
                    )
                    for sub in range(2):
                        qt = 2 * qtp + sub
                        for kti in range(4):
                            nc.tensor.transpose(
                                pPT[:, 512 * sub + 128 * kti : 512 * sub + 128 * (kti + 1)],
                                expT[(hh, kti)][:, ts(qt, 128)],
                                ident,
                            )
                    for sub in range(2):
                        qt = 2 * qtp + sub
                        nc.vector.tensor_scalar_mul(
                            pn[:, ts(qt, L)], pPT[:, ts(sub, 512)],
                            rc[:, 4 * qt + hh : 4 * qt + hh + 1],
                        )
                for qtp in range(2):
                    nc.sync.dma_start(
                        out=p_out[4 * hq + hh][ts(qtp, 256), :].rearrange(
                            "(a p) k -> p a k", p=128
                        ),
                        in_=pn[:, ts(qtp, 2 * L)].rearrange("p (a k) -> p a k", k=L),
                    )

            # 6) ctx transposes + normalize + DMA
            co_w = [
                copool.tile([128, 256], F32, name=f"co{br}_{hq}_{qt}", tag="co")
                for qt in range(4)
            ]
            for hh in range(4):
                pt = pspool.tile([128, 256], F16, name=f"psT{br}_{hq}_{hh}", tag="ps")
                for qt in range(4):
                    nc.tensor.transpose(
                        pt[:, ts(qt, 64)], csbs[hh][:, ts(qt, 128)],
                        ident[0:64, 0:64],
                    )
                for qt in range(4):
                    if qt % 2 == 0:
                        nc.vector.tensor_scalar_mul(
                            co_w[qt][:, ts(hh, 64)], pt[:, ts(qt, 64)],
                            rc[:, 4 * qt + hh : 4 * qt + hh + 1],
                        )
                    else:
                        nc.scalar.activation(
                            co_w[qt][:, ts(hh, 64)], pt[:, ts(qt, 64)],
                            mybir.ActivationFunctionType.Copy,
                            scale=rc[:, 4 * qt + hh : 4 * qt + hh + 1],
                        )
            for qt in range(4):
                nc.sync.dma_start(
                    out=c_out[ts(qt, 128), ts(hq, 256)], in_=co_w[qt]
                )

        pend = []
        for i in range(12):
            pend.append((i, emit_b_phase(i)))
            if len(pend) == 3:
                j, e = pend.pop(0)
                emit_post_phase(j, e)
        for j, e in pend:
            emit_post_phase(j, e)


def build_bass():
    nc = bacc.Bacc(
        "TRN2", target_bir_lowering=False, debug=False, num_devices=NCORES
    )
    ins = {}
    for m in range(3):
        ins[f"xt{m}"] = nc.dram_tensor(f"xt{m}", [D, L], F16, kind="ExternalInput").ap()
        for w in ("wqa", "wqb", "wk", "wv"):
            ins[f"{w}{m}"] = nc.dram_tensor(
                f"{w}{m}", [D, DKV], F16, kind="ExternalInput"
            ).ap()
    outs = {}
    for br in range(3):
        outs[f"p{br}"] = nc.dram_tensor(
            f"p{br}", [H, L, L], F32, kind="ExternalOutput"
        ).ap()
        outs[f"c{br}"] = nc.dram_tensor(
            f"c{br}", [L, D], F32, kind="ExternalOutput"
        ).ap()
    with tile.TileContext(nc) as tc:
        _build_kernel(tc, outs, ins)
    nc.compile()
    return nc


def _get_compiled():
    global _COMPILED
    if _COMPILED is None:
        _COMPILED = build_bass()
    return _COMPILED


def _prep_in_map(inputs, b):
    """Host-side marshalling for one core's batch element."""
    in_map = {}
    for m in range(3):
        x = np.asarray(inputs[f"input_tensor{m + 1}"][b], dtype=np.float32).reshape(
            L, D
        )
        in_map[f"xt{m}"] = np.ascontiguousarray(x.T).astype(np.float16)
        wq = np.asarray(inputs[f"w_q{m + 1}"], dtype=np.float32).reshape(D, H, 2, 32)
        in_map[f"wqa{m}"] = np.ascontiguousarray(
            wq[:, :, 0, :].reshape(D, DKV)
        ).astype(np.float16)
        in_map[f"wqb{m}"] = np.ascontiguousarray(
            wq[:, :, 1, :].reshape(D, DKV)
        ).astype(np.float16)
        in_map[f"wk{m}"] = np.asarray(inputs[f"w_k{m + 1}"]).astype(np.float16)
        in_map[f"wv{m}"] = np.asarray(inputs[f"w_v{m + 1}"]).astype(np.float16)
    return in_map


def _numpy_fallback(inputs):
    """Exact fp32 reference on host (only used if masks/biases nonzero)."""
    x = [np.asarray(inputs[f"input_tensor{i}"], np.float32).reshape(B, L, D)
         for i in (1, 2, 3)]
    msk = [np.asarray(inputs[f"attention_mask{i}"], np.float32).reshape(B, 1, 1, L)
           for i in (1, 2, 3)]

    def heads(t, hs):
        b, l, d = t.shape
        return t.reshape(b, l, H, hs).transpose(0, 2, 1, 3)

    q, k, v = [], [], []
    for i in range(3):
        wq = np.asarray(inputs[f"w_q{i + 1}"], np.float32)
        bq = np.asarray(inputs[f"b_q{i + 1}"], np.float32)
        wk = np.asarray(inputs[f"w_k{i + 1}"], np.float32)
        bk = np.asarray(inputs[f"b_k{i + 1}"], np.float32)
        wv = np.asarray(inputs[f"w_v{i + 1}"], np.float32)
        bv = np.asarray(inputs[f"b_v{i + 1}"], np.float32)
        q.append(heads(x[i] @ wq + bq, DH))
        k.append(heads(x[i] @ wk + bk, 32))
        v.append(heads(x[i] @ wv + bv, 32))

    def branch(qh, ka, kb, va, vb, bias):
        kc = np.concatenate([ka, kb], axis=-1)
        vc = np.concatenate([va, vb], axis=-1)
        s = np.einsum("bhqd,bhkd->bhqk", qh, kc) * SCALE + bias
        s = s - s.max(axis=-1, keepdims=True)
        e = np.exp(s)
        p = e / e.sum(axis=-1, keepdims=True)
        c = np.einsum("bhqk,bhkd->bhqd", p, vc)
        c = c.transpose(0, 2, 1, 3).reshape(B, T, S, D)
        return c.astype(np.float32), p.astype(np.float32)

    c1, p1 = branch(q[1], k[0], k[2], v[0], v[2], msk[0] + msk[2])
    c2, p2 = branch(q[0], k[1], k[2], v[1], v[2], 0.0)
    c3, p3 = branch(q[2], k[0], k[1], v[0], v[1], msk[0] + msk[1])
    return (c2, c1, c3, (p2, p1, p3))


def kernel(**inputs):
    # The fast path folds zero masks/biases into the kernel; verify and fall
    # back to an exact host computation if that assumption ever breaks.
    zeros_ok = all(
        not np.any(np.asarray(inputs[n]))
        for n in (
            "attention_mask1", "attention_mask2", "attention_mask3",
            "b_q1", "b_k1", "b_v1", "b_q2", "b_k2", "b_v2", "b_q3", "b_k3", "b_v3",
        )
        if n in inputs
    )
    if not zeros_ok:
        return _numpy_fallback(inputs)

    nc = _get_compiled()
    in_maps = [_prep_in_map(inputs, b) for b in range(NCORES)]
    res = run_bass_kernel_spmd(nc, in_maps, core_ids=list(range(NCORES)))
    return _assemble(res.results)


def _assemble(results):
    c = []
    p = []
    for br in range(3):
        cb = np.stack([results[b][f"c{br}"] for b in range(NCORES)])  # [8, 512, 1024]
        c.append(cb.reshape(B, T, S, D).astype(np.float32))
        pb = np.stack([results[b][f"p{br}"] for b in range(NCORES)])
        p.append(pb.astype(np.float32))
    return (c[1], c[0], c[2], (p[1], p[0], p[2]))


def run_traced(inputs, tmpdir=None):
    """For test.py: run with NTFF tracing, return (outputs, BassKernelResults)."""
    nc = _get_compiled()
    in_maps = [_prep_in_map(inputs, b) for b in range(NCORES)]
    res = run_bass_kernel_spmd(
        nc, in_maps, core_ids=list(range(NCORES)), trace=True, tmpdir=tmpdir
    )
    return _assemble(res.results), res


# revision 19
# speedup vs baseline: 1.1394x; 1.1394x over previous
"""BertBiAttention (3-modality co-attention) Trainium2 Bass kernel.

Sharding: data-parallel over batch B=8 across the 8 NeuronCores — each core
computes one batch element end-to-end (no collectives).

Per-core strategy (fp16 matmul inputs, fp32 accumulation):
  - Host pre-transposes/casts inputs: XT[m] = x_m.T  [D=1024, L=512] fp16;
    wq is split into per-head-half wqa/wqb [D, 512] so QTa/QTb chunks hold
    each head's 32 q-dims at partition offset 32*(h%4), matching KT chunks.
  - Projections on PE: QTa/QTb = wqa/wqb^T @ x^T, KT[m] = wk^T @ x^T,
    V[m] natural = x @ wv (chunks [128, 512]); the D=1024 contraction is
    split into two K=64 row groups (0 and 64) accumulating into separate
    PSUM banks so consecutive matmuls' LDWEIGHTS/streams overlap; the two
    banks are summed during evacuation (ACT copy + DVE add).
  - Scores computed once, as scores^T[k,q] chunks, with K_c=32 row-tiled
    matmuls 4-way packed across heads (row groups 0/32/64/96, separate
    banks) -> ACT exp(SCALE*s) -> fp16 expT.
  - Softmax denominators: ones-vector matmuls over expT (col-packed),
    gathered and PE-transposed to [q, head]; one reciprocal per quad.
  - probs output: expT blocks PE-transposed back to [q, k] layout in PSUM
    (fp16), then one DVE pass applies the per-partition reciprocal and
    widens to fp32; one consolidated 512 KiB DMA per head-half.
  - VC[branch] = per-head [va|vb] interleave of V, built by GPSIMD strided
    copies; ctx^T per head = single 4-matmul accumulation group
    [K=128, M=64, N=512] (even heads col-group 0, odd col-group 64, own
    banks); PE-transpose [64,128] blocks back to natural [q, d]; the
    normalization rides the PSUM evacuation as a per-partition scalar.
  - Quads (branch, head-quad) are software-pipelined: B-phase(i) is
    emitted before post-phase(i-1) so the PE works on quad i's scores
    while ACT computes quad i-1's exponentials.

Masks and biases are zero for this problem instance (per its input spec);
a numpy fallback handles any nonzero masks/biases just in case.
"""

import math
import os
import sys

import numpy as np

sys.path.insert(0, "/opt/trn_rl_repo")

from contextlib import ExitStack

import concourse.bass as bass
import concourse.tile as tile
from concourse import bacc, mybir
from concourse.bass import ts
from concourse.bass_utils import run_bass_kernel_spmd
from concourse.masks import make_identity

B, T, S = 8, 8, 64
L = T * S  # 512
D = 1024
H = 16
DH = 64  # q head size
DKV = 512  # k/v projection width (32 per head)
SCALE = 1.0 / math.sqrt(DH)
NCORES = 8

F16 = mybir.dt.float16
F32 = mybir.dt.float32
EXP = mybir.ActivationFunctionType.Exp

# (q_mod, a_mod, b_mod) per branch; reference returns
# (c[1], c[0], c[2], (p[1], p[0], p[2])) in this numbering.
BRANCHES = [(1, 0, 2), (0, 1, 2), (2, 0, 1)]

_COMPILED = None


def _build_kernel(tc, outs, ins):
    nc = tc.nc
    with ExitStack() as ctx:
        const = ctx.enter_context(tc.tile_pool(name="const", bufs=1))
        xpool = ctx.enter_context(tc.tile_pool(name="xpool", bufs=1))
        wpool = ctx.enter_context(tc.tile_pool(name="wpool", bufs=18))
        proj = ctx.enter_context(tc.tile_pool(name="proj", bufs=1))
        vcpool = ctx.enter_context(tc.tile_pool(name="vcpool", bufs=12))
        bpool = ctx.enter_context(tc.tile_pool(name="bpool", bufs=32))
        srpool = ctx.enter_context(tc.tile_pool(name="srpool", bufs=3))
        rpool = ctx.enter_context(tc.tile_pool(name="rpool", bufs=6))
        opool = ctx.enter_context(tc.tile_pool(name="opool", bufs=2))
        cpool = ctx.enter_context(tc.tile_pool(name="cpool", bufs=6))
        copool = ctx.enter_context(tc.tile_pool(name="copool", bufs=8))
        evpool = ctx.enter_context(tc.tile_pool(name="evpool", bufs=4))
        pspool = ctx.enter_context(tc.tile_pool(name="ps", bufs=8, space="PSUM"))

        ident = const.tile([128, 128], F16, name="ident", tag="ident")
        make_identity(nc, ident)
        ones = const.tile([128, 1], F16, name="ones", tag="ones")
        ident32 = const.tile([128, 128], F32, name="ident32", tag="ident32")
        make_identity(nc, ident32)
        nc.vector.memset(ones, 1.0)

        # ---- load XT chunks (resident); input loads trigger on gpsimd ----
        xt = [None] * 3  # xt[m][dc] : [128, 512] f16, rows = D dims dc*128..

        def _xt_load(m):
            if xt[m] is None:
                chunks = []
                for dc in range(8):
                    t = xpool.tile(
                        [128, L], F16, name=f"xt{m}_{dc}", tag=f"xt{m}_{dc}"
                    )
                    eng = nc.gpsimd if dc % 2 == 0 else nc.sync
                    eng.dma_start(out=t, in_=ins[f"xt{m}"][ts(dc, 128), :])
                    chunks.append(t)
                xt[m] = chunks

        # ---- projections (c-outer, dc-inner: 2 live banks, smooth evac) ----
        qta = [[None] * 4 for _ in range(3)]  # [m][hq] -> [128, 512] f16
        qtb = [[None] * 4 for _ in range(3)]
        kt = [[None] * 4 for _ in range(3)]  # K^T chunks
        vn = [[None] * 4 for _ in range(3)]  # V natural chunks [128 l, 512 dkv]

        def _wload(kind, m):
            tiles = []
            for dc in range(8):
                t = wpool.tile([128, DKV], F16, name=f"{kind}{m}_{dc}", tag="w")
                eng = nc.gpsimd if dc % 2 == 0 else nc.sync
                eng.dma_start(out=t, in_=ins[f"{kind}{m}"][ts(dc, 128), :])
                tiles.append(t)
            return tiles

        def _ksplit_mms(out_lo, out_hi, lhs_of, rhs_of):
            # contraction over 16 K=64 chunks: even halves -> row group 0
            # (bank out_lo), odd halves -> row group 64 (bank out_hi); the
            # alternating row groups let LDWEIGHTS/matmul streams overlap.
            for dc in range(8):
                st, sp = dc == 0, dc == 7
                nc.tensor.matmul(
                    out_lo, lhsT=lhs_of(dc, 0, 64), rhs=rhs_of(dc, 0, 64),
                    start=st, stop=sp, tile_position=(0, 0),
                )
                nc.tensor.matmul(
                    out_hi, lhsT=lhs_of(dc, 64, 128), rhs=rhs_of(dc, 64, 128),
                    start=st, stop=sp, tile_position=(64, 0),
                )

        def _evac_add(dst, p_lo, p_hi):
            tmp = evpool.tile([128, p_hi.shape[1]], F32, name="evtmp", tag="evtmp")
            nc.scalar.copy(tmp, p_hi)
            nc.vector.scalar_tensor_tensor(
                dst, p_lo, 1.0, tmp,
                op0=mybir.AluOpType.mult, op1=mybir.AluOpType.add,
            )

        def emit_q_proj(m):
            _xt_load(m)
            wa = _wload("wqa", m)
            wb = _wload("wqb", m)
            for c in range(4):
                qa_lo = pspool.tile([128, L], F32, name=f"qa_lo{m}_{c}", tag="ps")
                qa_hi = pspool.tile([128, L], F32, name=f"qa_hi{m}_{c}", tag="ps")
                qb_lo = pspool.tile([128, L], F32, name=f"qb_lo{m}_{c}", tag="ps")
                qb_hi = pspool.tile([128, L], F32, name=f"qb_hi{m}_{c}", tag="ps")
                _ksplit_mms(
                    qa_lo, qa_hi,
                    lambda dc, a, b: wa[dc][a:b, ts(c, 128)],
                    lambda dc, a, b: xt[m][dc][a:b, :],
                )
                _ksplit_mms(
                    qb_lo, qb_hi,
                    lambda dc, a, b: wb[dc][a:b, ts(c, 128)],
                    lambda dc, a, b: xt[m][dc][a:b, :],
                )
                ta = proj.tile([128, L], F16, name=f"qta{m}_{c}", tag=f"qta{m}_{c}")
                _evac_add(ta, qa_lo, qa_hi)
                qta[m][c] = ta
                tb = proj.tile([128, L], F16, name=f"qtb{m}_{c}", tag=f"qtb{m}_{c}")
                _evac_add(tb, qb_lo, qb_hi)
                qtb[m][c] = tb

        def emit_kv_proj(m):
            _xt_load(m)
            wk = _wload("wk", m)
            wv = _wload("wv", m)
            for c in range(4):
                k_lo = pspool.tile([128, L], F32, name=f"k_lo{m}_{c}", tag="ps")
                k_hi = pspool.tile([128, L], F32, name=f"k_hi{m}_{c}", tag="ps")
                v_lo = pspool.tile([128, DKV], F32, name=f"v_lo{m}_{c}", tag="ps")
                v_hi = pspool.tile([128, DKV], F32, name=f"v_hi{m}_{c}", tag="ps")
                _ksplit_mms(
                    k_lo, k_hi,
                    lambda dc, a, b: wk[dc][a:b, ts(c, 128)],
                    lambda dc, a, b: xt[m][dc][a:b, :],
                )
                _ksplit_mms(
                    v_lo, v_hi,
                    lambda dc, a, b: xt[m][dc][a:b, ts(c, 128)],
                    lambda dc, a, b: wv[dc][a:b, :],
                )
                tk = proj.tile([128, L], F16, name=f"kt{m}_{c}", tag=f"kt{m}_{c}")
                _evac_add(tk, k_lo, k_hi)
                kt[m][c] = tk
                tv = proj.tile([128, DKV], F16, name=f"vn{m}_{c}", tag=f"vn{m}_{c}")
                _evac_add(tv, v_lo, v_hi)
                vn[m][c] = tv

        # branch0 needs q:mod1, kv:mod0, kv:mod2 -> emit those first so
        # attention overlaps the remaining projections.
        emit_q_proj(1)
        emit_kv_proj(0)
        emit_kv_proj(2)
        emit_kv_proj(1)
        emit_q_proj(0)
        emit_q_proj(2)

        # ---- attention: software-pipelined quads across branches ----
        # quad index i = 4*br + hq; emit B-phase(i) then post-phase(i-1).
        vc_by_br = {}

        def emit_vc(br):
            qm, am, bm = BRANCHES[br]
            vcs = []
            for lc in range(4):
                v = vcpool.tile([128, D], F16, name=f"vc{br}_{lc}", tag="vc")
                vr = v.rearrange("p (h t j) -> p h t j", t=2, j=32)
                nc.gpsimd.tensor_copy(
                    vr[:, :, 0, :], vn[am][lc].rearrange("p (h j) -> p h j", j=32)
                )
                nc.gpsimd.tensor_copy(
                    vr[:, :, 1, :], vn[bm][lc].rearrange("p (h j) -> p h j", j=32)
                )
                vcs.append(v)
            vc_by_br[br] = vcs

        def emit_b_phase(i):
            br, hq = divmod(i, 4)
            qm, am, bm = BRANCHES[br]
            if hq == 0:
                emit_vc(br)
            expT = {}
            for kti in range(4):
                psb = []
                for hh in range(4):
                    off = 32 * hh
                    pb = pspool.tile(
                        [128, L], F32, name=f"psB{br}_{hq}_{kti}_{hh}", tag="ps"
                    )
                    nc.tensor.matmul(
                        pb,
                        lhsT=kt[am][hq][off : off + 32, ts(kti, 128)],
                        rhs=qta[qm][hq][off : off + 32, :],
                        start=True, stop=False, tile_position=(off, 0),
                    )
                    nc.tensor.matmul(
                        pb,
                        lhsT=kt[bm][hq][off : off + 32, ts(kti, 128)],
                        rhs=qtb[qm][hq][off : off + 32, :],
                        start=False, stop=True, tile_position=(off, 0),
                    )
                    psb.append(pb)
                for hh in range(4):
                    e = bpool.tile(
                        [128, L], F16, name=f"expT{br}_{hq}_{kti}_{hh}", tag="expT"
                    )
                    nc.scalar.activation(e, psb[hh], EXP, scale=SCALE)
                    expT[(hh, kti)] = e
            return expT

        def emit_post_phase(i, expT):
            br, hq = divmod(i, 4)
            qm, am, bm = BRANCHES[br]
            vc = vc_by_br[br]
            p_out = outs[f"p{br}"]
            c_out = outs[f"c{br}"]

            # 1) softmax denominators: sum_k expT via ones-matmuls
            sum_ps = []
            for hh in range(4):
                sp_t = pspool.tile([128, L], F32, name=f"psS{br}_{hq}_{hh}", tag="ps")
                sum_ps.append(sp_t)
            for kti in range(4):
                for hh in range(4):
                    nc.tensor.matmul(
                        sum_ps[hh][32 * hh : 32 * hh + 1, :],
                        lhsT=ones,
                        rhs=expT[(hh, kti)],
                        start=(kti == 0), stop=(kti == 3),
                        tile_position=(0, 32 * hh),
                    )
            # 2) gather rows (ACT) so PE isn't gated on the DVE queue
            srows4 = srpool.tile([128, L], F32, name=f"srows4{br}_{hq}", tag="srows4")
            for hh in range(4):
                nc.scalar.copy(
                    srows4[32 * hh : 32 * hh + 1, :],
                    sum_ps[hh][32 * hh : 32 * hh + 1, :],
                )

            # 3) ctx^T matmuls (independent of sums -> keeps PE busy)
            pcs = {}
            for hp in range(2):
                for sub in range(2):
                    hh = 2 * hp + sub
                    h = 4 * hq + hh
                    lo = 64 * sub
                    pc = pspool.tile(
                        [128, L], F32, name=f"psC{br}_{hq}_{hh}", tag="ps"
                    )
                    pcs[hh] = pc
                    for kti in range(4):
                        nc.tensor.matmul(
                            pc[lo : lo + 64, :],
                            lhsT=vc[kti][:, ts(h, 64)],
                            rhs=expT[(hh, kti)],
                            start=(kti == 0), stop=(kti == 3),
                            tile_position=(0, lo),
                        )
            csbs = {}
            for hh in range(4):
                lo = 64 * (hh % 2)
                csb = cpool.tile([64, L], F16, name=f"csb{br}_{hq}_{hh}", tag="csb")
                if hh % 2 == 0:
                    nc.vector.tensor_copy(csb, pcs[hh][lo : lo + 64, :])
                else:
                    nc.scalar.copy(csb, pcs[hh][lo : lo + 64, :])
                csbs[hh] = csb

            # 4) sums transpose + reciprocal
            psT4 = pspool.tile([128, L], F32, name=f"psT4{br}_{hq}", tag="ps")
            for qt in range(4):
                nc.tensor.transpose(
                    psT4[:, ts(qt, 128)], srows4[:, ts(qt, 128)], ident32
                )
            # rc[:, 4*qt + hh] = 1 / sum for q-rows of tile qt, head hh
            rc = rpool.tile([128, 16], F32, name=f"rc{br}_{hq}", tag="rc")
            nc.vector.reciprocal(
                rc.rearrange("p (a e j) -> p a e j", a=4, e=4, j=1),
                psT4.rearrange("p (a e j) -> p a e j", a=4, j=32)[:, :, :, 0:1],
            )

            # 5) probs: PE-transpose expT blocks, normalize+evac, 1 DMA/head
            for hh in range(4):
                pn = opool.tile([128, 4 * L], F32, name=f"pn{br}_{hq}_{hh}", tag="pn")
                for qtp in range(2):
                    pPT = pspool.tile(
                        [128, 1024], F16, name=f"psPT{br}_{hq}_{hh}_{qtp}", tag="ps"
                    )
                    for sub in range(2):
                        qt = 2 * qtp + sub
                        for kti in range(4):
                            nc.tensor.transpose(
                                pPT[:, 512 * sub + 128 * kti : 512 * sub + 128 * (kti + 1)],
                                expT[(hh, kti)][:, ts(qt, 128)],
                                ident,
                            )
                    for sub in range(2):
                        qt = 2 * qtp + sub
                        nc.vector.tensor_scalar_mul(
                            pn[:, ts(qt, L)], pPT[:, ts(sub, 512)],
                            rc[:, 4 * qt + hh : 4 * qt + hh + 1],
                        )
                for qtp in range(2):
                    nc.sync.dma_start(
                        out=p_out[4 * hq + hh][ts(qtp, 256), :].rearrange(
                            "(a p) k -> p a k", p=128
                        ),
                        in_=pn[:, ts(qtp, 2 * L)].rearrange("p (a k) -> p a k", k=L),
                    )

            # 6) ctx transposes + normalize + DMA
            co_w = [
                copool.tile([128, 256], F32, name=f"co{br}_{hq}_{qt}", tag="co")
                for qt in range(4)
            ]
            for hh in range(4):
                pt = pspool.tile([128, 256], F16, name=f"psT{br}_{hq}_{hh}", tag="ps")
                for qt in range(4):
                    nc.tensor.transpose(
                        pt[:, ts(qt, 64)], csbs[hh][:, ts(qt, 128)],
                        ident[0:64, 0:64],
                    )
                for qt in range(4):
                    if qt % 2 == 0:
                        nc.vector.tensor_scalar_mul(
                            co_w[qt][:, ts(hh, 64)], pt[:, ts(qt, 64)],
                            rc[:, 4 * qt + hh : 4 * qt + hh + 1],
                        )
                    else:
                        nc.scalar.activation(
                            co_w[qt][:, ts(hh, 64)], pt[:, ts(qt, 64)],
                            mybir.ActivationFunctionType.Copy,
                            scale=rc[:, 4 * qt + hh : 4 * qt + hh + 1],
                        )
            for qt in range(4):
                nc.sync.dma_start(
                    out=c_out[ts(qt, 128), ts(hq, 256)], in_=co_w[qt]
                )

        prev = None
        for i in range(12):
            cur = emit_b_phase(i)
            if prev is not None:
                emit_post_phase(i - 1, prev)
            prev = cur
        emit_post_phase(11, prev)


def build_bass():
    nc = bacc.Bacc(
        "TRN2", target_bir_lowering=False, debug=False, num_devices=NCORES
    )
    ins = {}
    for m in range(3):
        ins[f"xt{m}"] = nc.dram_tensor(f"xt{m}", [D, L], F16, kind="ExternalInput").ap()
        for w in ("wqa", "wqb", "wk", "wv"):
            ins[f"{w}{m}"] = nc.dram_tensor(
                f"{w}{m}", [D, DKV], F16, kind="ExternalInput"
            ).ap()
    outs = {}
    for br in range(3):
        outs[f"p{br}"] = nc.dram_tensor(
            f"p{br}", [H, L, L], F32, kind="ExternalOutput"
        ).ap()
        outs[f"c{br}"] = nc.dram_tensor(
            f"c{br}", [L, D], F32, kind="ExternalOutput"
        ).ap()
    with tile.TileContext(nc) as tc:
        _build_kernel(tc, outs, ins)
    nc.compile()
    return nc


def _get_compiled():
    global _COMPILED
    if _COMPILED is None:
        _COMPILED = build_bass()
    return _COMPILED


def _prep_in_map(inputs, b):
    """Host-side marshalling for one core's batch element."""
    in_map = {}
    for m in range(3):
        x = np.asarray(inputs[f"input_tensor{m + 1}"][b], dtype=np.float32).reshape(
            L, D
        )
        in_map[f"xt{m}"] = np.ascontiguousarray(x.T).astype(np.float16)
        wq = np.asarray(inputs[f"w_q{m + 1}"], dtype=np.float32).reshape(D, H, 2, 32)
        in_map[f"wqa{m}"] = np.ascontiguousarray(
            wq[:, :, 0, :].reshape(D, DKV)
        ).astype(np.float16)
        in_map[f"wqb{m}"] = np.ascontiguousarray(
            wq[:, :, 1, :].reshape(D, DKV)
        ).astype(np.float16)
        in_map[f"wk{m}"] = np.asarray(inputs[f"w_k{m + 1}"]).astype(np.float16)
        in_map[f"wv{m}"] = np.asarray(inputs[f"w_v{m + 1}"]).astype(np.float16)
    return in_map


def _numpy_fallback(inputs):
    """Exact fp32 reference on host (only used if masks/biases nonzero)."""
    x = [np.asarray(inputs[f"input_tensor{i}"], np.float32).reshape(B, L, D)
         for i in (1, 2, 3)]
    msk = [np.asarray(inputs[f"attention_mask{i}"], np.float32).reshape(B, 1, 1, L)
           for i in (1, 2, 3)]

    def heads(t, hs):
        b, l, d = t.shape
        return t.reshape(b, l, H, hs).transpose(0, 2, 1, 3)

    q, k, v = [], [], []
    for i in range(3):
        wq = np.asarray(inputs[f"w_q{i + 1}"], np.float32)
        bq = np.asarray(inputs[f"b_q{i + 1}"], np.float32)
        wk = np.asarray(inputs[f"w_k{i + 1}"], np.float32)
        bk = np.asarray(inputs[f"b_k{i + 1}"], np.float32)
        wv = np.asarray(inputs[f"w_v{i + 1}"], np.float32)
        bv = np.asarray(inputs[f"b_v{i + 1}"], np.float32)
        q.append(heads(x[i] @ wq + bq, DH))
        k.append(heads(x[i] @ wk + bk, 32))
        v.append(heads(x[i] @ wv + bv, 32))

    def branch(qh, ka, kb, va, vb, bias):
        kc = np.concatenate([ka, kb], axis=-1)
        vc = np.concatenate([va, vb], axis=-1)
        s = np.einsum("bhqd,bhkd->bhqk", qh, kc) * SCALE + bias
        s = s - s.max(axis=-1, keepdims=True)
        e = np.exp(s)
        p = e / e.sum(axis=-1, keepdims=True)
        c = np.einsum("bhqk,bhkd->bhqd", p, vc)
        c = c.transpose(0, 2, 1, 3).reshape(B, T, S, D)
        return c.astype(np.float32), p.astype(np.float32)

    c1, p1 = branch(q[1], k[0], k[2], v[0], v[2], msk[0] + msk[2])
    c2, p2 = branch(q[0], k[1], k[2], v[1], v[2], 0.0)
    c3, p3 = branch(q[2], k[0], k[1], v[0], v[1], msk[0] + msk[1])
    return (c2, c1, c3, (p2, p1, p3))


def kernel(**inputs):
    # The fast path folds zero masks/biases into the kernel; verify and fall
    # back to an exact host computation if that assumption ever breaks.
    zeros_ok = all(
        not np.any(np.asarray(inputs[n]))
        for n in (
            "attention_mask1", "attention_mask2", "attention_mask3",
            "b_q1", "b_k1", "b_v1", "b_q2", "b_k2", "b_v2", "b_q3", "b_k3", "b_v3",
        )
        if n in inputs
    )
    if not zeros_ok:
        return _numpy_fallback(inputs)

    nc = _get_compiled()
    in_maps = [_prep_in_map(inputs, b) for b in range(NCORES)]
    res = run_bass_kernel_spmd(nc, in_maps, core_ids=list(range(NCORES)))
    return _assemble(res.results)


def _assemble(results):
    c = []
    p = []
    for br in range(3):
        cb = np.stack([results[b][f"c{br}"] for b in range(NCORES)])  # [8, 512, 1024]
        c.append(cb.reshape(B, T, S, D).astype(np.float32))
        pb = np.stack([results[b][f"p{br}"] for b in range(NCORES)])
        p.append(pb.astype(np.float32))
    return (c[1], c[0], c[2], (p[1], p[0], p[2]))


def run_traced(inputs, tmpdir=None):
    """For test.py: run with NTFF tracing, return (outputs, BassKernelResults)."""
    nc = _get_compiled()
    in_maps = [_prep_in_map(inputs, b) for b in range(NCORES)]
    res = run_bass_kernel_spmd(
        nc, in_maps, core_ids=list(range(NCORES)), trace=True, tmpdir=tmpdir
    )
    return _assemble(res.results), res
